# revision 15
# baseline (speedup 1.0000x reference)
"""Trainium2 Bass kernel for nn_Class_Cross_Attention_V1 (B=4, N=196, Q=225, C=512, H=8).

Sharding: 8 cores = (batch b in 0..3) x (head-group hg in 0..1).
Each core: cross-attention + conv_ffn for its 4 heads / 256 channels,
AllGather (per 128-ch half) of pooled conv features within the pair,
then MAB + output projections for its n-half (98 rows).

v2 conv pipeline:
  - dw1 folded with the outer product hs = attn*v into a single banded
    matmul: out[c,q] (per n) = sum_{r=(i,j)} alpha[r,c] * at3[r,n,q]
    where alpha[r,c] = dw1[c,i,j]*v[n+i-1,c] (9-row stationary) and
    at3[r,n,q] = attn[h,q+j-1,n+i-1] (moving, n/q on the free axis).
    This removes the hs materialization, the 29MB broadcast DMA, and
    cuts dw1 TensorE time ~4x (225 cols/n vs 9 taps * spatial).
  - dw2 split between TensorE (diag-stationary, 9 shifted matmuls per
    row-pair) and VectorE (scalar_tensor_tensor FMA per tap).
  - BN+ReLU fused into ScalarE activation; pool via small DVE adds.
  - Per-pg AllGather overlaps the second head-group's conv.
"""

import sys
import os

sys.path.insert(0, "/opt/trn_rl_repo")

import numpy as np
import ml_dtypes

BF16 = ml_dtypes.bfloat16

# ---- problem constants (hardcoded; kernel.py must be self-contained) ----
B = 4
DIM = 512
H = 8
QL = 225                # cls tokens
N = 196                 # voxel_size
SEQ = N + QL            # 421
HD = DIM // H           # 64
EPS = 1e-5

QP = 228                # padded row width in h1/h2 tiles
ATC = 248               # aT tile cols: [0]=0, 1..226 data+pad, rest junk/zero
S_OUT = 14              # h2 rows per chunk
NCHUNK = N // S_OUT     # 14
LN = S_OUT + 2          # h1 rows per chunk (halo 1): ln 0..15 <-> n = 14c-1+ln
PE_ROWS = 8             # h2 rows per chunk on TensorE (rest on VectorE)
DVE_ROWS = S_OUT - PE_ROWS

NHALF = N // 2          # 98 output rows per core
H1LEN = LN * QP


def _build_program(sim_mode=False):
    import concourse.bass as bass
    import concourse.bacc as bacc
    import concourse.tile as tile
    from concourse import mybir

    f32 = mybir.dt.float32
    bf16 = mybir.dt.bfloat16
    AF = mybir.ActivationFunctionType
    OP = mybir.AluOpType

    nc = bacc.Bacc(None, target_bir_lowering=False, num_devices=8)

    # round-robin the two HWDGE queues (SP + Activation)
    dma_rr = [0]

    def dma(out, in_, **kw):
        eng = nc.sync if (dma_rr[0] % 2 == 0) else nc.scalar
        dma_rr[0] += 1
        return eng.dma_start(out=out, in_=in_, **kw)

    # ------------------- I/O (host-prearranged flat layouts) -------------------
    def inp(name, shape, dt=f32):
        return nc.dram_tensor(name, list(shape), dt, kind="ExternalInput")

    x_sb_d = inp("x_sb", [128, 4, SEQ], bf16)       # x[b].T as [p, kt, s]
    wq_d = inp("wq", [128, 4, 2, 128], bf16)        # (Wq.T*scale)[p,kt,mt,m]
    wk_d = inp("wk", [128, 4, 2, 128], bf16)
    wv_d = inp("wv", [128, 4, 2, 128], bf16)
    dw1r_d = inp("dw1r", [1, 9, 256], bf16)         # dw1 taps [r=3i+j, c-local]
    d2diag_d = inp("d2diag", [128, 2, 9, 128], bf16)
    w2sc_d = inp("w2sc", [128, 2, 9])               # dw2 taps per-partition
    s1t1_d = inp("s1t1", [128, 2, 2])               # [p][scale/bias][pg]
    s2t2_d = inp("s2t2", [128, 2, 2])
    clsT_d = inp("clsT", [128, 4, QL])              # f32 cls.T
    semTh_d = inp("semTh", [128, 4, NHALF])         # f32 sem-half.T
    mWq_d = inp("mWq", [128, 4, 4, 128])
    mWk_d = inp("mWk", [128, 4, 4, 128])            # pre-scaled 1/sqrt(512)
    mWo_d = inp("mWo", [128, 4, 4, 128])
    Wproj_d = inp("Wproj", [128, 4, 4, 128])
    pw_d = inp("pw", [128, 4, 4, 128])              # (pw/196).T
    mWv_d = inp("mWv", [128, 4, DIM])
    mbq_d = inp("mbq", [128, 4])
    mbk_d = inp("mbk", [128, 4])                    # pre-scaled
    mbo_d = inp("mbo", [128, 4])
    bproj_d = inp("bproj", [128, 4])
    mbv_d = inp("mbv", [1, DIM])
    ident_d = inp("ident", [128, 128])

    outT = nc.dram_tensor("outT", [DIM, NHALF], f32, kind="ExternalOutput")

    # internal DRAM for the two per-pg collectives
    S_in = [nc.dram_tensor(f"S_in{pg}", [128 * QL], f32) for pg in range(2)]
    S_out = [nc.dram_tensor(f"S_out{pg}", [2 * 128 * QL], f32) for pg in range(2)]

    with tile.TileContext(nc) as tc:
        with tc.tile_pool(name="persist", bufs=1) as persist:
            # ---------- persistent loads ----------
            x_sb = persist.tile([128, 4, SEQ], bf16)
            dma(x_sb[:], x_sb_d.ap())
            wq_sb = persist.tile([128, 4, 2, 128], bf16)
            wk_sb = persist.tile([128, 4, 2, 128], bf16)
            wv_sb = persist.tile([128, 4, 2, 128], bf16)
            dma(wq_sb[:], wq_d.ap())
            dma(wk_sb[:], wk_d.ap())
            dma(wv_sb[:], wv_d.ap())
            d2_sb = persist.tile([128, 2, 9, 128], bf16)
            dma(d2_sb[:], d2diag_d.ap())
            w2_sb = persist.tile([128, 2, 9], f32)
            dma(w2_sb[:], w2sc_d.ap())
            s1_sb = persist.tile([128, 2, 2], f32)
            s2_sb = persist.tile([128, 2, 2], f32)
            dma(s1_sb[:], s1t1_d.ap())
            dma(s2_sb[:], s2t2_d.ap())
            dw1r_sb = persist.tile([1, 9, 256], bf16)
            dma(dw1r_sb[:], dw1r_d.ap())
            ident_sb = persist.tile([128, 128], f32)
            dma(ident_sb[:], ident_d.ap())
            clsT_sb = persist.tile([128, 4, QL], f32)
            dma(clsT_sb[:], clsT_d.ap())
            semTh_sb = persist.tile([128, 4, NHALF], f32)
            dma(semTh_sb[:], semTh_d.ap())
            mbq_sb = persist.tile([128, 4], f32)
            mbk_sb = persist.tile([128, 4], f32)
            mbo_sb = persist.tile([128, 4], f32)
            bproj_sb = persist.tile([128, 4], f32)
            for t_, dr in ((mbq_sb, mbq_d), (mbk_sb, mbk_d),
                           (mbo_sb, mbo_d), (bproj_sb, bproj_d)):
                dma(t_[:], dr.ap())
            mbv_sb = persist.tile([1, DIM], f32)
            dma(mbv_sb[:], mbv_d.ap())
            ones_sb = persist.tile([1, 128], f32)
            nc.vector.memset(ones_sb[:], 1.0)

            # dw1 taps broadcast across partitions (for sv = v * tap)
            dw1rep = persist.tile([128, 9, 256], bf16)
            nc.gpsimd.partition_broadcast(dw1rep[:], dw1r_sb[:])

            # persistent stage A/B results
            qT_bf = persist.tile([128, 2, QL], bf16)
            kT_bf = persist.tile([128, 2, N], bf16)
            v_nc = [persist.tile([128, 256], f32, name="v_nc0"),
                    persist.tile([69, 256], f32, name="v_nc1")]
            sv_nc = [persist.tile([128, 9, 256], bf16, name="sv_nc0"),
                     persist.tile([69, 9, 256], bf16, name="sv_nc1")]
            aT = [[persist.tile([128, ATC], bf16, name=f"aT_{h}_{i}")
                   for i in range(2)] for h in range(4)]
            pool_acc = persist.tile([128, 2, QL], f32)
            nc.vector.memset(pool_acc[:], 0.0)

            # ---------- stage A: QKV + softmax + transposes ----------
            with (
                tc.tile_pool(name="stA", bufs=4) as stA,
                tc.tile_pool(name="stAp", bufs=2, space="PSUM") as stAp,
                tc.tile_pool(name="stApv", bufs=2, space="PSUM") as stApv,
            ):
                for mt in range(2):
                    pq = stAp.tile([128, QL], f32, tag="pq")
                    pk = stAp.tile([128, N], f32, tag="pk")
                    for kt in range(4):
                        fl = dict(start=(kt == 0), stop=(kt == 3))
                        nc.tensor.matmul(pq[:], wq_sb[:, kt, mt, :],
                                         x_sb[:, kt, N:SEQ], **fl)
                        nc.tensor.matmul(pk[:], wk_sb[:, kt, mt, :],
                                         x_sb[:, kt, 0:N], **fl)
                    nc.scalar.activation(qT_bf[:, mt, :], pq[:], AF.Copy)
                    nc.scalar.activation(kT_bf[:, mt, :], pk[:], AF.Copy)

                # v in [n, c] orientation: stationary = semT chunk, moving = wv
                NB = (128, 68)
                for nb in range(2):
                    nbn = NB[nb]
                    pv = stApv.tile([128, 256], f32, tag="pv")
                    for kt in range(4):
                        nc.tensor.matmul(
                            pv[0:nbn, :],
                            x_sb[:, kt, nb * 128: nb * 128 + nbn],
                            wv_sb[:, kt, :, :].rearrange("p a b -> p (a b)"),
                            start=(kt == 0), stop=(kt == 3),
                        )
                    if nb == 1:
                        # zero row 68 (m=196) before ACT fills rows 0..67
                        nc.vector.memset(v_nc[1][64:69, :], 0.0)
                    nc.scalar.activation(v_nc[nb][0:nbn, :], pv[0:nbn, :], AF.Copy)

                # sv[r][n, c] = v[n, c] * dw1tap[r, c]
                for nb in range(2):
                    nbn = NB[nb] + (1 if nb == 1 else 0)
                    nc.vector.tensor_mul(
                        sv_nc[nb][0:nbn, :, :],
                        v_nc[nb][0:nbn, :].unsqueeze(1).to_broadcast([nbn, 9, 256]),
                        dw1rep[0:nbn, :, :],
                    )

                # scores + softmax (no max subtraction: |scores| small)
                QB = (128, 97)
                for h in range(4):
                    pr = 64 * (h % 2)
                    mt = h // 2
                    for i in range(2):
                        nc.vector.memset(aT[h][i][:, 0:1], 0.0)
                        nc.vector.memset(aT[h][i][:, 226:ATC], 0.0)
                    for qb in range(2):
                        qbn = QB[qb]
                        qpad = 128 if qb == 0 else 112
                        ps = stAp.tile([128, N], f32, tag="ps")
                        nc.tensor.matmul(
                            ps[0:qbn, :],
                            qT_bf[pr: pr + 64, mt, qb * 128: qb * 128 + qbn],
                            kT_bf[pr: pr + 64, mt, :],
                        )
                        ae = stA.tile([128, 256], bf16, tag="ae")
                        an = stA.tile([128, 256], bf16, tag="an")
                        ssum = stA.tile([128, 1], f32, tag="ssum")
                        # junk q rows + the n=196 col (future aT1 row 68) must
                        # be zero; activation overwrites the valid region after
                        if qbn < qpad:
                            nc.vector.memset(an[96:qpad, 0:256], 0.0)
                        nc.vector.memset(an[0:qpad, N:256], 0.0)
                        nc.scalar.activation(
                            ae[0:qbn, 0:N], ps[0:qbn, :], AF.Exp,
                            accum_out=ssum[0:qbn, :],
                        )
                        rs = stA.tile([128, 1], f32, tag="rs")
                        nc.vector.reciprocal(rs[0:qbn, :], ssum[0:qbn, :])
                        nc.scalar.activation(
                            an[0:qbn, 0:N], ae[0:qbn, 0:N], AF.Copy,
                            scale=rs[0:qbn, :],
                        )
                        for nb in range(2):
                            nc.sync.dma_start_transpose(
                                aT[h][nb][:, 1 + qb * 128: 1 + qb * 128 + qpad],
                                an[0:qpad, nb * 128: (nb + 1) * 128],
                            )

            # ---------- stage B: conv pipeline, pg-major ----------
            with (
                tc.tile_pool(name="at3p", bufs=4) as at3p,
                tc.tile_pool(name="alphap", bufs=2) as alphap,
                tc.tile_pool(name="h1p", bufs=3) as h1p,
                tc.tile_pool(name="convp", bufs=3) as convp,
                tc.tile_pool(name="dw1ps", bufs=4, space="PSUM") as dw1ps,
                tc.tile_pool(name="dw2ps", bufs=3, space="PSUM") as dw2ps,
            ):
                for pg in range(2):
                    for ch in range(NCHUNK):
                        n0 = 14 * ch - 1          # n for ln=0
                        ln_lo = 1 if ch == 0 else 0
                        ln_hi = LN - 1 if ch == NCHUNK - 1 else LN

                        # ---- construct at3 (per head) and alpha ----
                        at3 = [at3p.tile([9, LN, QP], bf16, tag=f"at3_{hh}",
                                         name=f"at3_{hh}")
                               for hh in range(2)]
                        for hh in range(2):
                            h = 2 * pg + hh
                            for d in range(3):
                                for j in range(3):
                                    r = 3 * d + j
                                    # at3[r, ln, 2+q] = attn[h, q+j-1, m],
                                    # m = n0 - 1 + d + ln
                                    m_lo = n0 - 1 + d
                                    lo = max(0, -m_lo)
                                    hi = min(LN, 197 - m_lo)
                                    a = lo
                                    while a < hi:
                                        m = m_lo + a
                                        if m < 128:
                                            cnt = min(hi - a, 128 - m)
                                            dma(
                                                at3[hh][r:r + 1, a:a + cnt, 2:227],
                                                aT[h][0][m:m + cnt, j:j + 225],
                                            )
                                        else:
                                            cnt = hi - a
                                            dma(
                                                at3[hh][r:r + 1, a:a + cnt, 2:227],
                                                aT[h][1][m - 128:m - 128 + cnt,
                                                         j:j + 225],
                                            )
                                        a += cnt
                            if ch == 0:
                                # m=-1 rows (d=0, ln=1) must be zero
                                nc.vector.memset(at3[hh][0:3, 1, :], 0.0)

                        alpha = alphap.tile([9, LN, 128], bf16, tag="alpha")
                        if ch == 0:
                            nc.vector.memset(alpha[0:9, 0:2, :], 0.0)
                        if ch == NCHUNK - 1:
                            nc.vector.memset(alpha[0:9, LN - 1:LN, :], 0.0)
                        for d in range(3):
                            for j in range(3):
                                r = 3 * d + j
                                m_lo = n0 - 1 + d
                                lo = max(0, -m_lo)
                                hi = min(LN, 197 - m_lo)
                                a = lo
                                while a < hi:
                                    m = m_lo + a
                                    if m < 128:
                                        cnt = min(hi - a, 128 - m)
                                        dma(
                                            alpha[r:r + 1, a:a + cnt, :],
                                            sv_nc[0][m:m + cnt, r,
                                                     pg * 128:pg * 128 + 128],
                                        )
                                    else:
                                        cnt = hi - a
                                        dma(
                                            alpha[r:r + 1, a:a + cnt, :],
                                            sv_nc[1][m - 128:m - 128 + cnt, r,
                                                     pg * 128:pg * 128 + 128],
                                        )
                                    a += cnt

                        # ---- dw1: banded matmuls, 2 per (ln, hh) pair ----
                        h1 = h1p.tile([128, 8 + H1LEN], bf16, tag="h1")
                        h1r = h1[:, 4:4 + H1LEN].rearrange("p (r q) -> p r q", q=QP)
                        nc.vector.memset(h1[:, 0:6], 0.0)
                        nc.vector.memset(
                            h1[:, 231:231 + 15 * QP].rearrange(
                                "p (r q) -> p r q", q=QP)[:, :, 0:3], 0.0)
                        nc.vector.memset(h1[:, 3651:3656], 0.0)
                        if ch == 0:
                            nc.vector.memset(h1r[:, 0, :], 0.0)
                        if ch == NCHUNK - 1:
                            nc.vector.memset(h1r[:, LN - 1, :], 0.0)
                        ln = ln_lo
                        while ln < ln_hi:
                            nr = min(2, ln_hi - ln)
                            pw1 = dw1ps.tile([128, 2 * QP], f32, tag="pw1")
                            for nl in range(nr):
                                for hh in range(2):
                                    nc.tensor.matmul(
                                        pw1[64 * hh:64 * hh + 64,
                                            nl * QP: nl * QP + 225],
                                        alpha[0:9, ln + nl,
                                              64 * hh:64 * hh + 64],
                                        at3[hh][0:9, ln + nl, 2:227],
                                        start=True, stop=True,
                                        skip_group_check=True,
                                    )
                            nc.scalar.activation(
                                h1r[:, ln:ln + nr, 2:227],
                                pw1[:, 0:nr * QP].rearrange(
                                    "p (r q) -> p r q", q=QP)[:, :, 0:225],
                                AF.Relu,
                                scale=s1_sb[:, 0, pg:pg + 1],
                                bias=s1_sb[:, 1, pg:pg + 1],
                            )
                            ln += nr

                        # ---- dw2 rows [0, PE_ROWS) on TensorE ----
                        h2 = convp.tile([128, 8 + S_OUT * QP], bf16, tag="h2")
                        h2r = h2[:, 4:4 + S_OUT * QP].rearrange(
                            "p (r q) -> p r q", q=QP)
                        for r in range(0, PE_ROWS, 2):
                            W = 2 * QP
                            pw2 = dw2ps.tile([128, 2 * QP], f32, tag="pw2")
                            t = 0
                            for i in (-1, 0, 1):
                                for j in (-1, 0, 1):
                                    off = 4 + r * QP + QP * (1 + i) + j
                                    nc.tensor.matmul(
                                        pw2[:, 0:W],
                                        d2_sb[:, pg, t, :],
                                        h1[:, off: off + W],
                                        start=(t == 0), stop=(t == 8),
                                    )
                                    t += 1
                            nc.scalar.activation(
                                h2r[:, r:r + 2, 2:227],
                                pw2[:, 0:W].rearrange(
                                    "p (r q) -> p r q", q=QP)[:, :, 2:227],
                                AF.Relu,
                                scale=s2_sb[:, 0, pg:pg + 1],
                                bias=s2_sb[:, 1, pg:pg + 1],
                            )

                        # ---- dw2 rows [PE_ROWS, S_OUT) on VectorE ----
                        base = PE_ROWS * QP
                        cp_len = (DVE_ROWS + 2) * QP + 4
                        h1s = convp.tile([128, cp_len], bf16, tag="h1s")
                        dma(h1s[:, 0:cp_len],
                            h1[:, 4 + base - 1: 4 + base - 1 + cp_len])
                        acc = convp.tile([128, DVE_ROWS * QP], bf16, tag="acc")
                        L = DVE_ROWS * QP
                        t = 0
                        for i in (-1, 0, 1):
                            for j in (-1, 0, 1):
                                if j == 0:
                                    o = 4 + base + QP * (1 + i)
                                    sap = h1[:, o: o + L]
                                else:
                                    so = QP * (1 + i) + j + 1
                                    sap = h1s[:, so: so + L]
                                if t == 0:
                                    nc.vector.tensor_scalar_mul(
                                        acc[:, 0:L], sap, w2_sb[:, pg, t:t + 1])
                                else:
                                    nc.vector.scalar_tensor_tensor(
                                        acc[:, 0:L], sap, w2_sb[:, pg, t:t + 1],
                                        acc[:, 0:L],
                                        OP.mult, OP.add,
                                    )
                                t += 1
                        nc.scalar.activation(
                            h2r[:, PE_ROWS:S_OUT, 2:227],
                            acc[:, 0:L].rearrange(
                                "p (r q) -> p r q", q=QP)[:, :, 2:227],
                            AF.Relu,
                            scale=s2_sb[:, 0, pg:pg + 1],
                            bias=s2_sb[:, 1, pg:pg + 1],
                        )

                        # ---- pool: sum 14 h2 rows into pool_acc[pg] ----
                        p7 = convp.tile([128, 7, QL], bf16, tag="p7")
                        nc.vector.tensor_add(
                            p7[:], h2r[:, 0:7, 2:227], h2r[:, 7:14, 2:227])
                        p3 = convp.tile([128, 3, QL], bf16, tag="p3")
                        nc.vector.tensor_add(
                            p3[:], p7[:, 0:3, :], p7[:, 3:6, :])
                        pa = convp.tile([128, QL], f32, tag="pa")
                        nc.vector.tensor_add(pa[:], p3[:, 0, :], p3[:, 1, :])
                        pb = convp.tile([128, QL], f32, tag="pb")
                        nc.vector.tensor_add(pb[:], p3[:, 2, :], p7[:, 6, :])
                        pc = convp.tile([128, QL], f32, tag="pc")
                        nc.vector.tensor_add(pc[:], pa[:], pb[:])
                        nc.vector.tensor_add(
                            pool_acc[:, pg, :], pool_acc[:, pg, :], pc[:])

                    # ---- per-pg collective ----
                    dma(S_in[pg].ap().rearrange("(p q) -> p q", p=128),
                        pool_acc[:, pg, :])
                    if sim_mode:
                        half = 128 * QL
                        nc.sync.dma_start(out=S_out[pg].ap()[0:half],
                                          in_=S_in[pg].ap())
                        nc.sync.dma_start(out=S_out[pg].ap()[half:2 * half],
                                          in_=S_in[pg].ap())
                    else:
                        nc.gpsimd.collective_compute(
                            "AllGather",
                            mybir.AluOpType.bypass,
                            replica_groups=[[0, 1], [2, 3], [4, 5], [6, 7]],
                            ins=[S_in[pg].ap()],
                            outs=[S_out[pg].ap()],
                        )

            # ---------- stage D: MAB + projections for this n-half ----------
            with (
                tc.tile_pool(name="stD", bufs=1) as stD,
                tc.tile_pool(name="stDb", bufs=4) as stDb,
                tc.tile_pool(name="stDp", bufs=2, space="PSUM") as stDp,
                tc.tile_pool(name="stDpv", bufs=2, space="PSUM") as stDpv,
                tc.tile_pool(name="stDpo", bufs=1, space="PSUM") as stDpo,
            ):
                def load_w(dram):
                    t = stD.tile([128, 4, 4, 128], f32, tag=dram.name + "_sb",
                                 name=dram.name + "_sb")
                    dma(t[:], dram.ap())
                    return t

                mWq_sb = load_w(mWq_d)
                mWk_sb = load_w(mWk_d)
                mWo_sb = load_w(mWo_d)
                Wproj_sb = load_w(Wproj_d)
                pw_sb = load_w(pw_d)
                mWv_sb = stD.tile([128, 4, DIM], f32)
                dma(mWv_sb[:], mWv_d.ap())

                # Qm depends only on sem -> compute while collectives finish
                QmT_sb = stD.tile([128, 4, NHALF], f32)
                for mt in range(4):
                    pq2 = stDp.tile([128, NHALF], f32, tag="dps", name="pq2")
                    for kt in range(4):
                        nc.tensor.matmul(
                            pq2[:], mWq_sb[:, kt, mt, :], semTh_sb[:, kt, :],
                            start=(kt == 0), stop=(kt == 3),
                        )
                    nc.scalar.activation(
                        QmT_sb[:, mt, :], pq2[:], AF.Identity,
                        bias=mbq_sb[:, mt:mt + 1],
                    )

                # S: kt0 <- S_out0[0], kt1 <- S_out1[0], kt2 <- S_out0[1], kt3 <- S_out1[1]
                S_sb = stD.tile([128, 4, QL], f32)
                for kt in range(4):
                    src = S_out[kt % 2].ap().rearrange(
                        "(a p q) -> a p q", p=128, q=QL)[kt // 2]
                    dma(S_sb[:, kt, :], src)

                kcT_sb = stD.tile([128, 4, QL], f32)
                KmT_sb = stD.tile([128, 4, QL], f32)
                for mt in range(4):
                    pc2 = stDp.tile([128, QL], f32, tag="dps")
                    for kt in range(4):
                        nc.tensor.matmul(
                            pc2[:], pw_sb[:, kt, mt, :], S_sb[:, kt, :],
                            start=(kt == 0), stop=(kt == 3),
                        )
                    nc.vector.tensor_add(kcT_sb[:, mt, :], pc2[:], clsT_sb[:, mt, :])
                for mt in range(4):
                    pk2 = stDp.tile([128, QL], f32, tag="dps")
                    for kt in range(4):
                        nc.tensor.matmul(
                            pk2[:], mWk_sb[:, kt, mt, :], kcT_sb[:, kt, :],
                            start=(kt == 0), stop=(kt == 3),
                        )
                    nc.scalar.activation(
                        KmT_sb[:, mt, :], pk2[:], AF.Identity,
                        bias=mbk_sb[:, mt:mt + 1],
                    )

                # Vm (rows = q') with bias via ones-row matmul
                QB2 = (128, 97)
                Vm_sb = [stD.tile([128, DIM], f32, tag=f"vm{qb}", name=f"vm{qb}")
                         for qb in range(2)]
                for qb in range(2):
                    qbn = QB2[qb]
                    pv2 = stDpv.tile([128, DIM], f32, tag="pv2")
                    for kt in range(4):
                        nc.tensor.matmul(
                            pv2[0:qbn, :],
                            kcT_sb[:, kt, qb * 128: qb * 128 + qbn],
                            mWv_sb[:, kt, :],
                            start=(kt == 0), stop=False,
                        )
                    nc.tensor.matmul(
                        pv2[0:qbn, :], ones_sb[0:1, 0:qbn], mbv_sb[0:1, :],
                        start=False, stop=True,
                    )
                    nc.scalar.activation(Vm_sb[qb][0:qbn, :], pv2[0:qbn, :], AF.Copy)

                # per-head attention, transpose+normalize via diag(recip) matmul
                OT_sb = stD.tile([128, 4, NHALF], f32)
                po = [stDpo.tile([128, NHALF], f32, tag=f"po{i}", name=f"po{i}")
                      for i in range(4)]
                for h in range(H):
                    pr = 64 * (h % 2)
                    mt = h // 2
                    ps2 = stDp.tile([128, QL], f32, tag="dps")
                    nc.tensor.matmul(
                        ps2[0:NHALF, :],
                        QmT_sb[pr: pr + 64, mt, :],
                        KmT_sb[pr: pr + 64, mt, :],
                    )
                    a2e = stDb.tile([128, QL], f32, tag="a2e")
                    s2s = stDb.tile([128, 1], f32, tag="s2s")
                    nc.scalar.activation(
                        a2e[0:NHALF, :], ps2[0:NHALF, :], AF.Exp,
                        accum_out=s2s[0:NHALF, :],
                    )
                    r2s = stDb.tile([128, 1], f32, tag="r2s")
                    nc.vector.reciprocal(r2s[0:NHALF, :], s2s[0:NHALF, :])
                    dg = stDb.tile([128, NHALF], f32, tag="dg")
                    nc.vector.tensor_mul(
                        dg[0:NHALF, :],
                        ident_sb[0:NHALF, 0:NHALF],
                        r2s[0:NHALF, :].to_broadcast([NHALF, NHALF]),
                    )
                    a2T = stDb.tile([128, 2, NHALF], f32, tag="a2T")
                    for qb in range(2):
                        qbn = QB2[qb]
                        pt2 = stDp.tile([128, NHALF], f32, tag="dps")
                        nc.tensor.matmul(
                            pt2[0:qbn, :],
                            a2e[0:NHALF, qb * 128: qb * 128 + qbn],
                            dg[0:NHALF, 0:NHALF],
                        )
                        nc.scalar.activation(a2T[0:qbn, qb, :], pt2[0:qbn, :], AF.Copy)
                    for qb in range(2):
                        qbn = QB2[qb]
                        nc.tensor.matmul(
                            po[mt][pr: pr + 64, :],
                            Vm_sb[qb][0:qbn, 64 * h: 64 * h + 64],
                            a2T[0:qbn, qb, :],
                            start=(qb == 0), stop=(qb == 1),
                            skip_group_check=True,
                        )
                for mt in range(4):
                    nc.vector.tensor_add(OT_sb[:, mt, :], po[mt][:], QmT_sb[:, mt, :])

                # O2 = O + relu(mWo @ O + mbo); out = Wproj @ O2 + bproj
                O2T_sb = stD.tile([128, 4, NHALF], f32)
                for mt in range(4):
                    prr = stDp.tile([128, NHALF], f32, tag="dps")
                    for kt in range(4):
                        nc.tensor.matmul(
                            prr[:], mWo_sb[:, kt, mt, :], OT_sb[:, kt, :],
                            start=(kt == 0), stop=(kt == 3),
                        )
                    rT = stDb.tile([128, NHALF], f32, tag="rT")
                    nc.scalar.activation(
                        rT[:], prr[:], AF.Relu, bias=mbo_sb[:, mt:mt + 1])
                    nc.vector.tensor_add(O2T_sb[:, mt, :], OT_sb[:, mt, :], rT[:])
                outT_sb = stD.tile([128, 4, NHALF], f32)
                for mt in range(4):
                    pf = stDp.tile([128, NHALF], f32, tag="dps")
                    for kt in range(4):
                        nc.tensor.matmul(
                            pf[:], Wproj_sb[:, kt, mt, :], O2T_sb[:, kt, :],
                            start=(kt == 0), stop=(kt == 3),
                        )
                    nc.scalar.activation(
                        outT_sb[:, mt, :], pf[:], AF.Identity,
                        bias=bproj_sb[:, mt:mt + 1],
                    )
                nc.sync.dma_start(
                    out=outT.ap().rearrange("(a p) n -> p a n", p=128),
                    in_=outT_sb[:],
                )

    nc.compile()
    return nc


_NC = None


def _get_nc():
    global _NC
    if _NC is None:
        _NC = _build_program()
    return _NC


def _prep_inputs(inputs):
    """Build the 8 per-core input maps (host-side numpy weight prep)."""
    f = lambda a: np.ascontiguousarray(a, dtype=np.float32)
    bf = lambda a: np.ascontiguousarray(np.asarray(a, dtype=np.float32).astype(BF16))
    x = f(inputs["x"])
    Wq, Wk, Wv = f(inputs["Wq"]), f(inputs["Wk"]), f(inputs["Wv"])
    dw1, dw2, pw = f(inputs["dw1"]), f(inputs["dw2"]), f(inputs["pw"])
    scale = HD ** -0.5

    def bnfold(g, b, m, v):
        s = f(inputs[g]) / np.sqrt(f(inputs[v]) + EPS)
        t = f(inputs[b]) - f(inputs[m]) * s
        return s, t

    s1, t1 = bnfold("bn1_g", "bn1_b", "bn1_m", "bn1_v")
    s2, t2 = bnfold("bn2_g", "bn2_b", "bn2_m", "bn2_v")

    mWq, mbq = f(inputs["mWq"]), f(inputs["mbq"])
    mWk = f(inputs["mWk"]) / np.sqrt(DIM)
    mbk = f(inputs["mbk"]) / np.sqrt(DIM)
    mWv, mbv = f(inputs["mWv"]), f(inputs["mbv"])
    mWo, mbo = f(inputs["mWo"]), f(inputs["mbo"])
    Wproj, bproj = f(inputs["Wproj"]), f(inputs["bproj"])

    def wlayout(WT, nb=4):
        # [512, M] -> [128, 4, M/128-blocks...] : value[p, kt, mt, m]
        M = WT.shape[1]
        return f(WT.reshape(4, 128, M // 128, 128).transpose(1, 0, 2, 3))

    common = {
        "mWq": wlayout(mWq.T), "mWk": wlayout(mWk.T), "mWo": wlayout(mWo.T),
        "Wproj": wlayout(Wproj.T), "pw": wlayout((pw / N).T),
        "mWv": f(mWv.T.reshape(4, 128, DIM).transpose(1, 0, 2)),
        "mbq": f(mbq.reshape(4, 128).T), "mbk": f(mbk.reshape(4, 128).T),
        "mbo": f(mbo.reshape(4, 128).T), "bproj": f(bproj.reshape(4, 128).T),
        "mbv": f(mbv.reshape(1, DIM)), "ident": f(np.eye(128)),
    }

    in_maps = []
    for core in range(8):
        b, hg = core // 2, core % 2
        ch0 = hg * 256
        xT = x[b].T                    # (512, 421)
        m = dict(common)
        m["x_sb"] = bf(xT.reshape(4, 128, SEQ).transpose(1, 0, 2))
        m["clsT"] = f(xT[:, N:].reshape(4, 128, QL).transpose(1, 0, 2))
        m["semTh"] = f(xT[:, hg * NHALF: hg * NHALF + NHALF]
                       .reshape(4, 128, NHALF).transpose(1, 0, 2))
        m["wq"] = bf((Wq.T[:, ch0:ch0 + 256] * scale)
                     .reshape(4, 128, 2, 128).transpose(1, 0, 2, 3))
        m["wk"] = bf(Wk.T[:, ch0:ch0 + 256]
                     .reshape(4, 128, 2, 128).transpose(1, 0, 2, 3))
        m["wv"] = bf(Wv.T[:, ch0:ch0 + 256]
                     .reshape(4, 128, 2, 128).transpose(1, 0, 2, 3))
        # dw1 taps [r = 3i+j, c-local]
        d1r = np.zeros((1, 9, 256), np.float32)
        for i in range(3):
            for j in range(3):
                d1r[0, 3 * i + j, :] = dw1[ch0:ch0 + 256, 0, i, j]
        m["dw1r"] = bf(d1r)
        d2 = np.zeros((2, 9, 128, 128), np.float32)
        w2s = np.zeros((2, 9, 128), np.float32)
        for pg in range(2):
            cs = ch0 + pg * 128
            for t, (i, j) in enumerate([(i, j) for i in range(3) for j in range(3)]):
                d2[pg, t, np.arange(128), np.arange(128)] = dw2[cs:cs + 128, 0, i, j]
                w2s[pg, t] = dw2[cs:cs + 128, 0, i, j]
        m["d2diag"] = bf(d2.transpose(2, 0, 1, 3))
        m["w2sc"] = f(w2s.transpose(2, 0, 1))
        m["s1t1"] = f(np.stack([s1[ch0:ch0 + 256].reshape(2, 128),
                                t1[ch0:ch0 + 256].reshape(2, 128)])
                      .transpose(2, 0, 1))
        m["s2t2"] = f(np.stack([s2[ch0:ch0 + 256].reshape(2, 128),
                                t2[ch0:ch0 + 256].reshape(2, 128)])
                      .transpose(2, 0, 1))
        in_maps.append(m)
    return in_maps


_LAST_RESULT = {"res": None}


def kernel(**inputs):
    from concourse.bass_utils import run_bass_kernel_spmd

    nc = _get_nc()
    in_maps = _prep_inputs(inputs)
    trace = bool(int(os.environ.get("KERNEL_TRACE", "0")))
    res = run_bass_kernel_spmd(nc, in_maps, core_ids=list(range(8)), trace=trace)
    _LAST_RESULT["res"] = res
    out = np.zeros((B, N, DIM), np.float32)
    for core in range(8):
        b, hg = core // 2, core % 2
        out[b, hg * NHALF: hg * NHALF + NHALF, :] = res.results[core]["outT"].T
    return out


# revision 22
# speedup vs baseline: 1.0198x; 1.0198x over previous
"""Trainium2 Bass kernel for nn_Class_Cross_Attention_V1 (B=4, N=196, Q=225, C=512, H=8).

Sharding: 8 cores = (batch b in 0..3) x (head-group hg in 0..1).
Each core: cross-attention + conv_ffn for its 4 heads / 256 channels,
AllGather (per 128-ch half) of pooled conv features within the pair,
then MAB + output projections for its n-half (98 rows).

v2 conv pipeline:
  - dw1 folded with the outer product hs = attn*v into a single banded
    matmul: out[c,q] (per n) = sum_{r=(i,j)} alpha[r,c] * at3[r,n,q]
    where alpha[r,c] = dw1[c,i,j]*v[n+i-1,c] (9-row stationary) and
    at3[r,n,q] = attn[h,q+j-1,n+i-1] (moving, n/q on the free axis).
    This removes the hs materialization, the 29MB broadcast DMA, and
    cuts dw1 TensorE time ~4x (225 cols/n vs 9 taps * spatial).
  - dw2 split between TensorE (diag-stationary, 9 shifted matmuls per
    row-pair) and VectorE (scalar_tensor_tensor FMA per tap).
  - BN+ReLU fused into ScalarE activation; pool via small DVE adds.
  - Per-pg AllGather overlaps the second head-group's conv.
"""

import sys
import os

sys.path.insert(0, "/opt/trn_rl_repo")

import numpy as np
import ml_dtypes

BF16 = ml_dtypes.bfloat16

# ---- problem constants (hardcoded; kernel.py must be self-contained) ----
B = 4
DIM = 512
H = 8
QL = 225                # cls tokens
N = 196                 # voxel_size
SEQ = N + QL            # 421
HD = DIM // H           # 64
EPS = 1e-5

QP = 228                # padded row width in h1/h2 tiles
ATC = 248               # aT tile cols: [0]=0, 1..226 data+pad, rest junk/zero
S_OUT = 14              # h2 rows per chunk
NCHUNK = N // S_OUT     # 14
LN = S_OUT + 2          # h1 rows per chunk (halo 1): ln 0..15 <-> n = 14c-1+ln
PE_ROWS = 8             # h2 rows per chunk on TensorE (rest on VectorE)
DVE_ROWS = S_OUT - PE_ROWS

NHALF = N // 2          # 98 output rows per core
H1LEN = LN * QP


def _build_program(sim_mode=False, debug=False):
    import concourse.bass as bass
    import concourse.bacc as bacc
    import concourse.tile as tile
    from concourse import mybir

    f32 = mybir.dt.float32
    bf16 = mybir.dt.bfloat16
    AF = mybir.ActivationFunctionType
    OP = mybir.AluOpType

    nc = bacc.Bacc(None, target_bir_lowering=False, num_devices=8)

    # round-robin the two HWDGE queues (SP + Activation)
    dma_rr = [0]

    def dma(out, in_, **kw):
        eng = nc.sync if (dma_rr[0] % 2 == 0) else nc.scalar
        dma_rr[0] += 1
        return eng.dma_start(out=out, in_=in_, **kw)

    # ------------------- I/O (host-prearranged flat layouts) -------------------
    def inp(name, shape, dt=f32):
        return nc.dram_tensor(name, list(shape), dt, kind="ExternalInput")

    x_sb_d = inp("x_sb", [128, 4, SEQ], bf16)       # x[b].T as [p, kt, s]
    wq_d = inp("wq", [128, 4, 2, 128], bf16)        # (Wq.T*scale)[p,kt,mt,m]
    wk_d = inp("wk", [128, 4, 2, 128], bf16)
    wv_d = inp("wv", [128, 4, 2, 128], bf16)
    dw1r_d = inp("dw1r", [1, 9, 256], bf16)         # dw1 taps [r=3i+j, c-local]
    d2diag_d = inp("d2diag", [128, 2, 9, 128], bf16)
    w2sc_d = inp("w2sc", [128, 2, 9])               # dw2 taps per-partition
    s1t1_d = inp("s1t1", [128, 2, 2])               # [p][scale/bias][pg]
    s2t2_d = inp("s2t2", [128, 2, 2])
    clsT_d = inp("clsT", [128, 4, QL])              # f32 cls.T
    semTh_d = inp("semTh", [128, 4, NHALF])         # f32 sem-half.T
    mWq_d = inp("mWq", [128, 4, 4, 128])
    mWk_d = inp("mWk", [128, 4, 4, 128])            # pre-scaled 1/sqrt(512)
    mWo_d = inp("mWo", [128, 4, 4, 128])
    Wproj_d = inp("Wproj", [128, 4, 4, 128])
    pw_d = inp("pw", [128, 4, 4, 128])              # (pw/196).T
    mWv_d = inp("mWv", [128, 4, DIM])
    mbq_d = inp("mbq", [128, 4])
    mbk_d = inp("mbk", [128, 4])                    # pre-scaled
    mbo_d = inp("mbo", [128, 4])
    bproj_d = inp("bproj", [128, 4])
    mbv_d = inp("mbv", [1, DIM])
    ident_d = inp("ident", [128, 128])

    outT = nc.dram_tensor("outT", [DIM, NHALF], f32, kind="ExternalOutput")
    if debug:
        dbg_rep = nc.dram_tensor("dbg_rep", [128, 9 * 256], f32, kind="ExternalOutput")
        dbg_alpha = nc.dram_tensor("dbg_alpha", [9, LN * 128], f32, kind="ExternalOutput")
        dbg_at3 = nc.dram_tensor("dbg_at3", [9, LN * QP], f32, kind="ExternalOutput")
        dbg_h1 = nc.dram_tensor("dbg_h1", [128, H1LEN], f32, kind="ExternalOutput")

    # internal DRAM for the two per-pg collectives
    S_in = [nc.dram_tensor(f"S_in{pg}", [128 * QL], f32) for pg in range(2)]
    S_out = [nc.dram_tensor(f"S_out{pg}", [2 * 128 * QL], f32) for pg in range(2)]
    # DRAM staging for the conv construction reads (rows 197..199 zero)
    aT_dram = nc.dram_tensor("aT_dram", [4, 200, QP], bf16)
    sv_dram = nc.dram_tensor("sv_dram", [9, 200, 256], bf16)

    with tile.TileContext(nc) as tc:
        with tc.tile_pool(name="persist", bufs=1) as persist:
            # ---------- persistent loads ----------
            x_sb = persist.tile([128, 4, SEQ], bf16)
            dma(x_sb[:], x_sb_d.ap())
            wq_sb = persist.tile([128, 4, 2, 128], bf16)
            wk_sb = persist.tile([128, 4, 2, 128], bf16)
            wv_sb = persist.tile([128, 4, 2, 128], bf16)
            dma(wq_sb[:], wq_d.ap())
            dma(wk_sb[:], wk_d.ap())
            dma(wv_sb[:], wv_d.ap())
            d2_sb = persist.tile([128, 2, 9, 128], bf16)
            dma(d2_sb[:], d2diag_d.ap())
            w2_sb = persist.tile([128, 2, 9], f32)
            dma(w2_sb[:], w2sc_d.ap())
            s1_sb = persist.tile([128, 2, 2], f32)
            s2_sb = persist.tile([128, 2, 2], f32)
            dma(s1_sb[:], s1t1_d.ap())
            dma(s2_sb[:], s2t2_d.ap())
            dw1r_sb = persist.tile([1, 9, 256], bf16)
            dma(dw1r_sb[:], dw1r_d.ap())
            ident_sb = persist.tile([128, 128], f32)
            dma(ident_sb[:], ident_d.ap())
            clsT_sb = persist.tile([128, 4, QL], f32)
            dma(clsT_sb[:], clsT_d.ap())
            semTh_sb = persist.tile([128, 4, NHALF], f32)
            dma(semTh_sb[:], semTh_d.ap())
            mbq_sb = persist.tile([128, 4], f32)
            mbk_sb = persist.tile([128, 4], f32)
            mbo_sb = persist.tile([128, 4], f32)
            bproj_sb = persist.tile([128, 4], f32)
            for t_, dr in ((mbq_sb, mbq_d), (mbk_sb, mbk_d),
                           (mbo_sb, mbo_d), (bproj_sb, bproj_d)):
                dma(t_[:], dr.ap())
            mbv_sb = persist.tile([1, DIM], f32)
            dma(mbv_sb[:], mbv_d.ap())
            ones_sb = persist.tile([1, 128], f32)
            nc.vector.memset(ones_sb[:], 1.0)

            # dw1 taps broadcast across partitions (for sv = v * tap)
            dw1rep = persist.tile([128, 9, 256], bf16)
            nc.gpsimd.partition_broadcast(dw1rep[:], dw1r_sb[:])

            if debug:
                nc.gpsimd.dma_start(
                    out=dbg_rep.ap(),
                    in_=dw1rep[:].rearrange("p a b -> p (a b)"))

            # persistent stage A/B results
            qT_bf = persist.tile([128, 2, QL], bf16)
            kT_bf = persist.tile([128, 2, N], bf16)
            v_nc = [persist.tile([128, 256], f32, name="v_nc0"),
                    persist.tile([69, 256], f32, name="v_nc1")]
            sv_nc = [persist.tile([128, 9, 256], bf16, name="sv_nc0"),
                     persist.tile([69, 9, 256], bf16, name="sv_nc1")]
            aT = [[persist.tile([128, ATC], bf16, name=f"aT_{h}_{i}")
                   for i in range(2)] for h in range(4)]
            pool_acc = persist.tile([128, 2, QL], f32)
            nc.vector.memset(pool_acc[:], 0.0)

            # ---------- stage A: QKV + softmax + transposes ----------
            with (
                tc.tile_pool(name="stA", bufs=4) as stA,
                tc.tile_pool(name="stAp", bufs=2, space="PSUM") as stAp,
                tc.tile_pool(name="stApv", bufs=2, space="PSUM") as stApv,
            ):
                for mt in range(2):
                    pq = stAp.tile([128, QL], f32, tag="pq")
                    pk = stAp.tile([128, N], f32, tag="pk")
                    for kt in range(4):
                        fl = dict(start=(kt == 0), stop=(kt == 3))
                        nc.tensor.matmul(pq[:], wq_sb[:, kt, mt, :],
                                         x_sb[:, kt, N:SEQ], **fl)
                        nc.tensor.matmul(pk[:], wk_sb[:, kt, mt, :],
                                         x_sb[:, kt, 0:N], **fl)
                    nc.scalar.activation(qT_bf[:, mt, :], pq[:], AF.Copy)
                    nc.scalar.activation(kT_bf[:, mt, :], pk[:], AF.Copy)

                # v in [n, c] orientation: stationary = semT chunk, moving = wv
                NB = (128, 68)
                for nb in range(2):
                    nbn = NB[nb]
                    pv = stApv.tile([128, 256], f32, tag="pv")
                    for kt in range(4):
                        nc.tensor.matmul(
                            pv[0:nbn, :],
                            x_sb[:, kt, nb * 128: nb * 128 + nbn],
                            wv_sb[:, kt, :, :].rearrange("p a b -> p (a b)"),
                            start=(kt == 0), stop=(kt == 3),
                        )
                    if nb == 1:
                        # zero row 68 (m=196) before ACT fills rows 0..67
                        nc.vector.memset(v_nc[1][64:69, :], 0.0)
                    nc.scalar.activation(v_nc[nb][0:nbn, :], pv[0:nbn, :], AF.Copy)

                # sv[r][n, c] = v[n, c] * dw1tap[r, c]
                for nb in range(2):
                    nbn = NB[nb] + (1 if nb == 1 else 0)
                    nc.vector.tensor_mul(
                        sv_nc[nb][0:nbn, :, :],
                        v_nc[nb][0:nbn, :].unsqueeze(1).to_broadcast([nbn, 9, 256]),
                        dw1rep[0:nbn, :, :],
                    )

                # scores + softmax (no max subtraction: |scores| small)
                QB = (128, 97)
                for h in range(4):
                    pr = 64 * (h % 2)
                    mt = h // 2
                    for i in range(2):
                        nc.vector.memset(aT[h][i][:, 0:1], 0.0)
                        nc.vector.memset(aT[h][i][:, 226:ATC], 0.0)
                    for qb in range(2):
                        qbn = QB[qb]
                        qpad = 128 if qb == 0 else 112
                        ps = stAp.tile([128, N], f32, tag="ps")
                        nc.tensor.matmul(
                            ps[0:qbn, :],
                            qT_bf[pr: pr + 64, mt, qb * 128: qb * 128 + qbn],
                            kT_bf[pr: pr + 64, mt, :],
                        )
                        ae = stA.tile([128, 256], bf16, tag="ae")
                        an = stA.tile([128, 256], bf16, tag="an")
                        ssum = stA.tile([128, 1], f32, tag="ssum")
                        # junk q rows + the n=196 col (future aT1 row 68) must
                        # be zero; activation overwrites the valid region after
                        if qbn < qpad:
                            nc.vector.memset(an[96:qpad, 0:256], 0.0)
                        nc.vector.memset(an[0:qpad, N:256], 0.0)
                        nc.scalar.activation(
                            ae[0:qbn, 0:N], ps[0:qbn, :], AF.Exp,
                            accum_out=ssum[0:qbn, :],
                        )
                        rs = stA.tile([128, 1], f32, tag="rs")
                        nc.vector.reciprocal(rs[0:qbn, :], ssum[0:qbn, :])
                        nc.scalar.activation(
                            an[0:qbn, 0:N], ae[0:qbn, 0:N], AF.Copy,
                            scale=rs[0:qbn, :],
                        )
                        for nb in range(2):
                            nc.sync.dma_start_transpose(
                                aT[h][nb][:, 1 + qb * 128: 1 + qb * 128 + qpad],
                                an[0:qpad, nb * 128: (nb + 1) * 128],
                            )

            # ---------- stage A->B staging: aT and sv to DRAM ----------
            zero3 = persist.tile([1, 768], bf16)
            nc.vector.memset(zero3[:], 0.0)
            for h in range(4):
                nc.sync.dma_start(out=aT_dram.ap()[h, 0:128, :],
                                  in_=aT[h][0][:, 0:QP])
                nc.sync.dma_start(out=aT_dram.ap()[h, 128:197, :],
                                  in_=aT[h][1][0:69, 0:QP])
                nc.scalar.dma_start(out=aT_dram.ap()[h, 197:200, :],
                                    in_=zero3[0:1, 0:684])
            for r in range(9):
                nc.sync.dma_start(out=sv_dram.ap()[r, 0:128, :],
                                  in_=sv_nc[0][:, r, :])
                nc.scalar.dma_start(out=sv_dram.ap()[r, 128:197, :],
                                    in_=sv_nc[1][0:69, r, :])
            for r in range(9):
                nc.sync.dma_start(out=sv_dram.ap()[r, 197:200, :],
                                  in_=zero3[0:1, 0:768])

            # ---------- stage B: conv pipeline, pg-major ----------
            with (
                tc.tile_pool(name="at3p", bufs=4) as at3p,
                tc.tile_pool(name="alphap", bufs=2) as alphap,
                tc.tile_pool(name="h1p", bufs=3) as h1p,
                tc.tile_pool(name="convp", bufs=3) as convp,
                tc.tile_pool(name="dw1ps", bufs=4, space="PSUM") as dw1ps,
                tc.tile_pool(name="dw2ps", bufs=3, space="PSUM") as dw2ps,
            ):
                for pg in range(2):
                    for ch in range(NCHUNK):
                        n0 = 14 * ch - 1          # n for ln=0
                        ln_lo = 1 if ch == 0 else 0
                        ln_hi = LN - 1 if ch == NCHUNK - 1 else LN

                        # ---- construct at3 (per head) and alpha ----
                        # at3[r=(d,j), ln, 2+q] = aT_dram[h, n0-1+d+ln, j+q]
                        at3 = [at3p.tile([9, LN, QP], bf16, tag=f"at3_{hh}",
                                         name=f"at3_{hh}")
                               for hh in range(2)]
                        for hh in range(2):
                            h = 2 * pg + hh
                            if ch == 0:
                                # m=-1 rows (d=0, ln<=1) must be zero
                                nc.vector.memset(at3[hh][0:9, 0:2, :], 0.0)
                            for d in range(3):
                                lo = max(0, 2 - d) if ch == 0 else 0
                                src = bass.AP(
                                    tensor=aT_dram,
                                    offset=(h * 200 + n0 - 1 + d + lo) * QP,
                                    ap=[[1, 3], [QP, LN - lo], [1, 225]],
                                )
                                nc.scalar.dma_start(
                                    out=at3[hh][3 * d:3 * d + 3, lo:LN, 2:227],
                                    in_=src)

                        # alpha[r=(i,j), ln, c] = sv_dram[r, n0+ln+i-1, c]
                        alpha = alphap.tile([9, LN, 128], bf16, tag="alpha")
                        if ch == 0:
                            nc.vector.memset(alpha[0:9, 0:2, :], 0.0)
                        for i in range(3):
                            lo = max(0, 2 - i) if ch == 0 else 0
                            src = bass.AP(
                                tensor=sv_dram,
                                offset=(3 * i * 200 + n0 + lo + i - 1) * 256
                                + pg * 128,
                                ap=[[200 * 256, 3], [256, LN - lo], [1, 128]],
                            )
                            nc.sync.dma_start(
                                out=alpha[3 * i:3 * i + 3, lo:LN, :], in_=src)

                        if debug and ch == 5 and pg == 0:
                            nc.gpsimd.dma_start(
                                out=dbg_alpha.ap(),
                                in_=alpha[:].rearrange("p a b -> p (a b)"))
                            nc.gpsimd.dma_start(
                                out=dbg_at3.ap(),
                                in_=at3[0][:].rearrange("p a b -> p (a b)"))

                        # ---- dw1: banded matmuls, 2 per (ln, hh) pair ----
                        h1 = h1p.tile([128, 8 + H1LEN], bf16, tag="h1")
                        h1r = h1[:, 4:4 + H1LEN].rearrange("p (r q) -> p r q", q=QP)
                        nc.vector.memset(h1[:, 0:6], 0.0)
                        nc.vector.memset(
                            h1[:, 231:231 + 15 * QP].rearrange(
                                "p (r q) -> p r q", q=QP)[:, :, 0:3], 0.0)
                        nc.vector.memset(h1[:, 3651:3656], 0.0)
                        if ch == 0:
                            nc.vector.memset(h1r[:, 0, :], 0.0)
                        if ch == NCHUNK - 1:
                            nc.vector.memset(h1r[:, LN - 1, :], 0.0)
                        ln = ln_lo
                        while ln < ln_hi:
                            nr = min(2, ln_hi - ln)
                            pw1 = dw1ps.tile([128, 2 * QP], f32, tag="pw1")
                            for nl in range(nr):
                                for hh in range(2):
                                    nc.tensor.matmul(
                                        pw1[64 * hh:64 * hh + 64,
                                            nl * QP: nl * QP + 225],
                                        alpha[0:9, ln + nl,
                                              64 * hh:64 * hh + 64],
                                        at3[hh][0:9, ln + nl, 2:227],
                                        start=True, stop=True,
                                        skip_group_check=True,
                                    )
                            nc.scalar.activation(
                                h1r[:, ln:ln + nr, 2:227],
                                pw1[:, 0:nr * QP].rearrange(
                                    "p (r q) -> p r q", q=QP)[:, :, 0:225],
                                AF.Relu,
                                scale=s1_sb[:, 0, pg:pg + 1],
                                bias=s1_sb[:, 1, pg:pg + 1],
                            )
                            ln += nr

                        if debug and ch == 5 and pg == 0:
                            nc.gpsimd.dma_start(
                                out=dbg_h1.ap(),
                                in_=h1[:, 4:4 + H1LEN])

                        # ---- dw2 rows [0, PE_ROWS) on TensorE ----
                        h2 = convp.tile([128, 8 + S_OUT * QP], bf16, tag="h2")
                        h2r = h2[:, 4:4 + S_OUT * QP].rearrange(
                            "p (r q) -> p r q", q=QP)
                        for r in range(0, PE_ROWS, 2):
                            W = 2 * QP
                            pw2 = dw2ps.tile([128, 2 * QP], f32, tag="pw2")
                            t = 0
                            for i in (-1, 0, 1):
                                for j in (-1, 0, 1):
                                    off = 4 + r * QP + QP * (1 + i) + j
                                    nc.tensor.matmul(
                                        pw2[:, 0:W],
                                        d2_sb[:, pg, t, :],
                                        h1[:, off: off + W],
                                        start=(t == 0), stop=(t == 8),
                                    )
                                    t += 1
                            nc.scalar.activation(
                                h2r[:, r:r + 2, 2:227],
                                pw2[:, 0:W].rearrange(
                                    "p (r q) -> p r q", q=QP)[:, :, 2:227],
                                AF.Relu,
                                scale=s2_sb[:, 0, pg:pg + 1],
                                bias=s2_sb[:, 1, pg:pg + 1],
                            )

                        # ---- dw2 rows [PE_ROWS, S_OUT) on VectorE ----
                        base = PE_ROWS * QP
                        cp_len = (DVE_ROWS + 2) * QP + 4
                        h1s = convp.tile([128, cp_len], bf16, tag="h1s")
                        nc.sync.dma_start(
                            out=h1s[:, 0:cp_len],
                            in_=h1[:, 4 + base - 1: 4 + base - 1 + cp_len])
                        acc = convp.tile([128, DVE_ROWS * QP], bf16, tag="acc")
                        L = DVE_ROWS * QP
                        t = 0
                        for i in (-1, 0, 1):
                            for j in (-1, 0, 1):
                                if j == 0:
                                    o = 4 + base + QP * (1 + i)
                                    sap = h1[:, o: o + L]
                                else:
                                    so = QP * (1 + i) + j + 1
                                    sap = h1s[:, so: so + L]
                                if t == 0:
                                    nc.vector.tensor_scalar_mul(
                                        acc[:, 0:L], sap, w2_sb[:, pg, t:t + 1])
                                else:
                                    nc.vector.scalar_tensor_tensor(
                                        acc[:, 0:L], sap, w2_sb[:, pg, t:t + 1],
                                        acc[:, 0:L],
                                        OP.mult, OP.add,
                                    )
                                t += 1
                        nc.scalar.activation(
                            h2r[:, PE_ROWS:S_OUT, 2:227],
                            acc[:, 0:L].rearrange(
                                "p (r q) -> p r q", q=QP)[:, :, 2:227],
                            AF.Relu,
                            scale=s2_sb[:, 0, pg:pg + 1],
                            bias=s2_sb[:, 1, pg:pg + 1],
                        )

                        # ---- pool: sum 14 h2 rows into pool_acc[pg] ----
                        p7 = convp.tile([128, 7, QL], bf16, tag="p7")
                        nc.vector.tensor_add(
                            p7[:], h2r[:, 0:7, 2:227], h2r[:, 7:14, 2:227])
                        p3 = convp.tile([128, 3, QL], bf16, tag="p3")
                        nc.vector.tensor_add(
                            p3[:], p7[:, 0:3, :], p7[:, 3:6, :])
                        pa = convp.tile([128, QL], f32, tag="pa")
                        nc.vector.tensor_add(pa[:], p3[:, 0, :], p3[:, 1, :])
                        pb = convp.tile([128, QL], f32, tag="pb")
                        nc.vector.tensor_add(pb[:], p3[:, 2, :], p7[:, 6, :])
                        pc = convp.tile([128, QL], f32, tag="pc")
                        nc.vector.tensor_add(pc[:], pa[:], pb[:])
                        nc.vector.tensor_add(
                            pool_acc[:, pg, :], pool_acc[:, pg, :], pc[:])

                    # ---- per-pg collective ----
                    dma(S_in[pg].ap().rearrange("(p q) -> p q", p=128),
                        pool_acc[:, pg, :])
                    if sim_mode:
                        half = 128 * QL
                        nc.sync.dma_start(out=S_out[pg].ap()[0:half],
                                          in_=S_in[pg].ap())
                        nc.sync.dma_start(out=S_out[pg].ap()[half:2 * half],
                                          in_=S_in[pg].ap())
                    else:
                        nc.gpsimd.collective_compute(
                            "AllGather",
                            mybir.AluOpType.bypass,
                            replica_groups=[[0, 1], [2, 3], [4, 5], [6, 7]],
                            ins=[S_in[pg].ap()],
                            outs=[S_out[pg].ap()],
                        )

            # ---------- stage D: MAB + projections for this n-half ----------
            with (
                tc.tile_pool(name="stD", bufs=1) as stD,
                tc.tile_pool(name="stDb", bufs=4) as stDb,
                tc.tile_pool(name="stDp", bufs=2, space="PSUM") as stDp,
                tc.tile_pool(name="stDpv", bufs=2, space="PSUM") as stDpv,
                tc.tile_pool(name="stDpo", bufs=1, space="PSUM") as stDpo,
            ):
                def load_w(dram):
                    t = stD.tile([128, 4, 4, 128], f32, tag=dram.name + "_sb",
                                 name=dram.name + "_sb")
                    dma(t[:], dram.ap())
                    return t

                mWq_sb = load_w(mWq_d)
                mWk_sb = load_w(mWk_d)
                mWo_sb = load_w(mWo_d)
                Wproj_sb = load_w(Wproj_d)
                pw_sb = load_w(pw_d)
                mWv_sb = stD.tile([128, 4, DIM], f32)
                dma(mWv_sb[:], mWv_d.ap())

                # Qm depends only on sem -> compute while collectives finish
                QmT_sb = stD.tile([128, 4, NHALF], f32)
                for mt in range(4):
                    pq2 = stDp.tile([128, NHALF], f32, tag="dps", name="pq2")
                    for kt in range(4):
                        nc.tensor.matmul(
                            pq2[:], mWq_sb[:, kt, mt, :], semTh_sb[:, kt, :],
                            start=(kt == 0), stop=(kt == 3),
                        )
                    nc.scalar.activation(
                        QmT_sb[:, mt, :], pq2[:], AF.Identity,
                        bias=mbq_sb[:, mt:mt + 1],
                    )

                # S: kt0 <- S_out0[0], kt1 <- S_out1[0], kt2 <- S_out0[1], kt3 <- S_out1[1]
                S_sb = stD.tile([128, 4, QL], f32)
                for kt in range(4):
                    src = S_out[kt % 2].ap().rearrange(
                        "(a p q) -> a p q", p=128, q=QL)[kt // 2]
                    dma(S_sb[:, kt, :], src)

                kcT_sb = stD.tile([128, 4, QL], f32)
                KmT_sb = stD.tile([128, 4, QL], f32)
                for mt in range(4):
                    pc2 = stDp.tile([128, QL], f32, tag="dps")
                    for kt in range(4):
                        nc.tensor.matmul(
                            pc2[:], pw_sb[:, kt, mt, :], S_sb[:, kt, :],
                            start=(kt == 0), stop=(kt == 3),
                        )
                    nc.vector.tensor_add(kcT_sb[:, mt, :], pc2[:], clsT_sb[:, mt, :])
                for mt in range(4):
                    pk2 = stDp.tile([128, QL], f32, tag="dps")
                    for kt in range(4):
                        nc.tensor.matmul(
                            pk2[:], mWk_sb[:, kt, mt, :], kcT_sb[:, kt, :],
                            start=(kt == 0), stop=(kt == 3),
                        )
                    nc.scalar.activation(
                        KmT_sb[:, mt, :], pk2[:], AF.Identity,
                        bias=mbk_sb[:, mt:mt + 1],
                    )

                # Vm (rows = q') with bias via ones-row matmul
                QB2 = (128, 97)
                Vm_sb = [stD.tile([128, DIM], f32, tag=f"vm{qb}", name=f"vm{qb}")
                         for qb in range(2)]
                for qb in range(2):
                    qbn = QB2[qb]
                    pv2 = stDpv.tile([128, DIM], f32, tag="pv2")
                    for kt in range(4):
                        nc.tensor.matmul(
                            pv2[0:qbn, :],
                            kcT_sb[:, kt, qb * 128: qb * 128 + qbn],
                            mWv_sb[:, kt, :],
                            start=(kt == 0), stop=False,
                        )
                    nc.tensor.matmul(
                        pv2[0:qbn, :], ones_sb[0:1, 0:qbn], mbv_sb[0:1, :],
                        start=False, stop=True,
                    )
                    nc.scalar.activation(Vm_sb[qb][0:qbn, :], pv2[0:qbn, :], AF.Copy)

                # per-head attention, transpose+normalize via diag(recip) matmul
                OT_sb = stD.tile([128, 4, NHALF], f32)
                po = [stDpo.tile([128, NHALF], f32, tag=f"po{i}", name=f"po{i}")
                      for i in range(4)]
                for h in range(H):
                    pr = 64 * (h % 2)
                    mt = h // 2
                    ps2 = stDp.tile([128, QL], f32, tag="dps")
                    nc.tensor.matmul(
                        ps2[0:NHALF, :],
                        QmT_sb[pr: pr + 64, mt, :],
                        KmT_sb[pr: pr + 64, mt, :],
                    )
                    a2e = stDb.tile([128, QL], f32, tag="a2e")
                    s2s = stDb.tile([128, 1], f32, tag="s2s")
                    nc.scalar.activation(
                        a2e[0:NHALF, :], ps2[0:NHALF, :], AF.Exp,
                        accum_out=s2s[0:NHALF, :],
                    )
                    r2s = stDb.tile([128, 1], f32, tag="r2s")
                    nc.vector.reciprocal(r2s[0:NHALF, :], s2s[0:NHALF, :])
                    dg = stDb.tile([128, NHALF], f32, tag="dg")
                    nc.vector.tensor_mul(
                        dg[0:NHALF, :],
                        ident_sb[0:NHALF, 0:NHALF],
                        r2s[0:NHALF, :].to_broadcast([NHALF, NHALF]),
                    )
                    a2T = stDb.tile([128, 2, NHALF], f32, tag="a2T")
                    for qb in range(2):
                        qbn = QB2[qb]
                        pt2 = stDp.tile([128, NHALF], f32, tag="dps")
                        nc.tensor.matmul(
                            pt2[0:qbn, :],
                            a2e[0:NHALF, qb * 128: qb * 128 + qbn],
                            dg[0:NHALF, 0:NHALF],
                        )
                        nc.scalar.activation(a2T[0:qbn, qb, :], pt2[0:qbn, :], AF.Copy)
                    for qb in range(2):
                        qbn = QB2[qb]
                        nc.tensor.matmul(
                            po[mt][pr: pr + 64, :],
                            Vm_sb[qb][0:qbn, 64 * h: 64 * h + 64],
                            a2T[0:qbn, qb, :],
                            start=(qb == 0), stop=(qb == 1),
                            skip_group_check=True,
                        )
                for mt in range(4):
                    nc.vector.tensor_add(OT_sb[:, mt, :], po[mt][:], QmT_sb[:, mt, :])

                # O2 = O + relu(mWo @ O + mbo); out = Wproj @ O2 + bproj
                O2T_sb = stD.tile([128, 4, NHALF], f32)
                for mt in range(4):
                    prr = stDp.tile([128, NHALF], f32, tag="dps")
                    for kt in range(4):
                        nc.tensor.matmul(
                            prr[:], mWo_sb[:, kt, mt, :], OT_sb[:, kt, :],
                            start=(kt == 0), stop=(kt == 3),
                        )
                    rT = stDb.tile([128, NHALF], f32, tag="rT")
                    nc.scalar.activation(
                        rT[:], prr[:], AF.Relu, bias=mbo_sb[:, mt:mt + 1])
                    nc.vector.tensor_add(O2T_sb[:, mt, :], OT_sb[:, mt, :], rT[:])
                outT_sb = stD.tile([128, 4, NHALF], f32)
                for mt in range(4):
                    pf = stDp.tile([128, NHALF], f32, tag="dps")
                    for kt in range(4):
                        nc.tensor.matmul(
                            pf[:], Wproj_sb[:, kt, mt, :], O2T_sb[:, kt, :],
                            start=(kt == 0), stop=(kt == 3),
                        )
                    nc.scalar.activation(
                        outT_sb[:, mt, :], pf[:], AF.Identity,
                        bias=bproj_sb[:, mt:mt + 1],
                    )
                nc.sync.dma_start(
                    out=outT.ap().rearrange("(a p) n -> p a n", p=128),
                    in_=outT_sb[:],
                )

    nc.compile()
    return nc


_NC = None


def _get_nc():
    global _NC
    if _NC is None:
        _NC = _build_program()
    return _NC


def _prep_inputs(inputs):
    """Build the 8 per-core input maps (host-side numpy weight prep)."""
    f = lambda a: np.ascontiguousarray(a, dtype=np.float32)
    bf = lambda a: np.ascontiguousarray(np.asarray(a, dtype=np.float32).astype(BF16))
    x = f(inputs["x"])
    Wq, Wk, Wv = f(inputs["Wq"]), f(inputs["Wk"]), f(inputs["Wv"])
    dw1, dw2, pw = f(inputs["dw1"]), f(inputs["dw2"]), f(inputs["pw"])
    scale = HD ** -0.5

    def bnfold(g, b, m, v):
        s = f(inputs[g]) / np.sqrt(f(inputs[v]) + EPS)
        t = f(inputs[b]) - f(inputs[m]) * s
        return s, t

    s1, t1 = bnfold("bn1_g", "bn1_b", "bn1_m", "bn1_v")
    s2, t2 = bnfold("bn2_g", "bn2_b", "bn2_m", "bn2_v")

    mWq, mbq = f(inputs["mWq"]), f(inputs["mbq"])
    mWk = f(inputs["mWk"]) / np.sqrt(DIM)
    mbk = f(inputs["mbk"]) / np.sqrt(DIM)
    mWv, mbv = f(inputs["mWv"]), f(inputs["mbv"])
    mWo, mbo = f(inputs["mWo"]), f(inputs["mbo"])
    Wproj, bproj = f(inputs["Wproj"]), f(inputs["bproj"])

    def wlayout(WT, nb=4):
        # [512, M] -> [128, 4, M/128-blocks...] : value[p, kt, mt, m]
        M = WT.shape[1]
        return f(WT.reshape(4, 128, M // 128, 128).transpose(1, 0, 2, 3))

    common = {
        "mWq": wlayout(mWq.T), "mWk": wlayout(mWk.T), "mWo": wlayout(mWo.T),
        "Wproj": wlayout(Wproj.T), "pw": wlayout((pw / N).T),
        "mWv": f(mWv.T.reshape(4, 128, DIM).transpose(1, 0, 2)),
        "mbq": f(mbq.reshape(4, 128).T), "mbk": f(mbk.reshape(4, 128).T),
        "mbo": f(mbo.reshape(4, 128).T), "bproj": f(bproj.reshape(4, 128).T),
        "mbv": f(mbv.reshape(1, DIM)), "ident": f(np.eye(128)),
    }

    in_maps = []
    for core in range(8):
        b, hg = core // 2, core % 2
        ch0 = hg * 256
        xT = x[b].T                    # (512, 421)
        m = dict(common)
        m["x_sb"] = bf(xT.reshape(4, 128, SEQ).transpose(1, 0, 2))
        m["clsT"] = f(xT[:, N:].reshape(4, 128, QL).transpose(1, 0, 2))
        m["semTh"] = f(xT[:, hg * NHALF: hg * NHALF + NHALF]
                       .reshape(4, 128, NHALF).transpose(1, 0, 2))
        m["wq"] = bf((Wq.T[:, ch0:ch0 + 256] * scale)
                     .reshape(4, 128, 2, 128).transpose(1, 0, 2, 3))
        m["wk"] = bf(Wk.T[:, ch0:ch0 + 256]
                     .reshape(4, 128, 2, 128).transpose(1, 0, 2, 3))
        m["wv"] = bf(Wv.T[:, ch0:ch0 + 256]
                     .reshape(4, 128, 2, 128).transpose(1, 0, 2, 3))
        # dw1 taps [r = 3i+j, c-local]
        d1r = np.zeros((1, 9, 256), np.float32)
        for i in range(3):
            for j in range(3):
                d1r[0, 3 * i + j, :] = dw1[ch0:ch0 + 256, 0, i, j]
        m["dw1r"] = bf(d1r)
        d2 = np.zeros((2, 9, 128, 128), np.float32)
        w2s = np.zeros((2, 9, 128), np.float32)
        for pg in range(2):
            cs = ch0 + pg * 128
            for t, (i, j) in enumerate([(i, j) for i in range(3) for j in range(3)]):
                d2[pg, t, np.arange(128), np.arange(128)] = dw2[cs:cs + 128, 0, i, j]
                w2s[pg, t] = dw2[cs:cs + 128, 0, i, j]
        m["d2diag"] = bf(d2.transpose(2, 0, 1, 3))
        m["w2sc"] = f(w2s.transpose(2, 0, 1))
        m["s1t1"] = f(np.stack([s1[ch0:ch0 + 256].reshape(2, 128),
                                t1[ch0:ch0 + 256].reshape(2, 128)])
                      .transpose(2, 0, 1))
        m["s2t2"] = f(np.stack([s2[ch0:ch0 + 256].reshape(2, 128),
                                t2[ch0:ch0 + 256].reshape(2, 128)])
                      .transpose(2, 0, 1))
        in_maps.append(m)
    return in_maps


_LAST_RESULT = {"res": None}


def kernel(**inputs):
    from concourse.bass_utils import run_bass_kernel_spmd

    nc = _get_nc()
    in_maps = _prep_inputs(inputs)
    trace = bool(int(os.environ.get("KERNEL_TRACE", "0")))
    res = run_bass_kernel_spmd(nc, in_maps, core_ids=list(range(8)), trace=trace)
    _LAST_RESULT["res"] = res
    out = np.zeros((B, N, DIM), np.float32)
    for core in range(8):
        b, hg = core // 2, core % 2
        out[b, hg * NHALF: hg * NHALF + NHALF, :] = res.results[core]["outT"].T
    return out
